# revision 7
# baseline (speedup 1.0000x reference)
"""BerHu loss kernel for Trainium2, 8-core data-parallel.

Reference computation (per sample n over its S = 1*480*640 elements):
    d  = pred - tgt
    c  = max|d| / 5
    berhu = |d|                 where |d| <= c
          = (d^2 + c^2) / (2c)  otherwise
    loss = mean_n mean_i berhu

Identity used on device:  berhu = |d| + relu(|d| - c)^2 * (1/(2c))
Two custom DVE ops do all heavy lifting (one pass each over the data):
  ABSDIFF:  ad = |p - t|            accum: mx = max(ad)      [per partition]
  BERHU:    junk = ad + relu(ad-c)^2 * i2c   accum: bh = sum [per partition]
The host sums the per-partition/per-sample bh partials:
    loss = sum(bh) / (N * S).

Sharding: pure data parallel, 8 samples per core on 8 cores; each
sample's 307200 elements are laid out [128 partitions x 2400].
"""

import numpy as np

N = 64          # batch
S = 307200      # 1*480*640 elements per sample
NCORES = 8
NLOC = N // NCORES   # samples per core
P = 128              # SBUF partitions
F = S // P           # 2400 columns per sample

_PROG = None


def _register_ops():
    import concourse.dve_ops as dve_ops
    from concourse.dve_ops import OPS, DveOp, has_src1
    from concourse.dve_spec import C0, C1, AluOp, Spec, Src0, Src1, Zero, lower
    from concourse.dve_spec import relu, sq, maxx
    from concourse.dve_uop import DveOpSpec

    def add_op(name, spec):
        for o in OPS:
            if o.name == name:
                return o
        op = DveOp(name, spec, subdim=False, uops_sha={})
        OPS.append(op)
        dve_ops.CUSTOM_DVE_SPECS[name] = spec
        dve_ops._SUB_OPCODE_FOR_NAME[name] = (
            dve_ops._CUSTOM_DVE_ROW_BASE + len(OPS) - 1)
        assert dve_ops._SUB_OPCODE_FOR_NAME[name] < 0x20
        for ver in ("v3", "v4"):
            sha = DveOpSpec(
                name=name,
                opcode=dve_ops.get_dve_sub_opcode(name),
                uops=lower(spec, ver=ver),
                rd1_en=has_src1(spec),
            ).sha(ver)
            op.uops_sha[ver] = sha
        return op

    def _absdiff_ref(in0, in1, c0, c1, c2):
        x = in0.astype(np.float32).reshape(in0.shape[0], -1)
        y = np.asarray(in1, np.float32).reshape(in0.shape[0], -1)
        out = np.abs(x - y).astype(np.float32)
        return out, out.max(axis=-1)

    def _berhu_ref(in0, in1, c0, c1, c2):
        x = in0.astype(np.float32).reshape(in0.shape[0], -1)
        r = np.maximum(x - c0, 0.0).astype(np.float32)
        out = (x + r * r * c1).astype(np.float32)
        return out, out.sum(axis=-1, dtype=np.float32)

    d = Src0 - Src1
    absdiff = add_op(
        "ANT_BERHU_ABSDIFF",
        Spec(body=maxx(d, Zero - d), accum=AluOp.MAX, reference=_absdiff_ref),
    )
    berhu = add_op(
        "ANT_BERHU_ACC",
        Spec(body=Src0 + sq(relu(Src0 - C0)) * C1, accum=AluOp.ADD,
             reference=_berhu_ref),
    )
    return absdiff, berhu


def _build(repeat=1):
    """Build the per-core program. `repeat` > 1 replays the whole 8-sample
    body that many times inside one NEFF (benchmarking only)."""
    from contextlib import ExitStack

    import concourse.bacc as bacc
    import concourse.tile as tile
    from concourse import mybir

    absdiff_op, berhu_op = _register_ops()

    f32 = mybir.dt.float32
    Alu = mybir.AluOpType

    nc = bacc.Bacc("TRN2", target_bir_lowering=False, debug=False,
                   num_devices=NCORES)
    p_d = nc.dram_tensor("p", [NLOC * P, F], f32, kind="ExternalInput").ap()
    t_d = nc.dram_tensor("t", [NLOC * P, F], f32, kind="ExternalInput").ap()
    bh_d = nc.dram_tensor("bh", [P, NLOC], f32, kind="ExternalOutput").ap()

    with tile.TileContext(nc) as tc, ExitStack() as ctx:
        io = ctx.enter_context(tc.tile_pool(name="io", bufs=4))
        work = ctx.enter_context(tc.tile_pool(name="work", bufs=3))
        small = ctx.enter_context(tc.tile_pool(name="small", bufs=3))
        stats = ctx.enter_context(tc.tile_pool(name="stats", bufs=1))

        bh_t = stats.tile([P, NLOC], f32, tag="bh")
        total = NLOC * repeat

        def load_pass1(i):
            n = i % NLOC
            rows = slice(n * P, (n + 1) * P)
            pt = io.tile([P, F], f32, tag="p")
            tt = io.tile([P, F], f32, tag="t")
            nc.sync.dma_start(out=pt[:], in_=p_d[rows, :])
            nc.sync.dma_start(out=tt[:], in_=t_d[rows, :])
            # ad = |p - t|; mxn = per-partition max(ad)
            ad = work.tile([P, F], f32, tag="ad")
            mxn = small.tile([P, 1], f32, tag="mxn")
            nc.vector._custom_dve(absdiff_op, out=ad[:], in0=pt[:], in1=tt[:],
                                  accum_out=mxn[:])
            return {"ad": ad, "mxn": mxn}

        def chain(st):
            # cross-partition max via tiny gather DMA on the ACT HWDGE ring,
            # then c = m/5 and i2c = 2.5/m, replicated and scattered
            mrow = small.tile([1, P], f32, tag="mrow")
            nc.scalar.dma_start(out=mrow[:], in_=st["mxn"][:])
            cc = small.tile([1, 2], f32, tag="cc")
            nc.vector.tensor_reduce(out=cc[0:1, 0:1], in_=mrow[:],
                                    axis=mybir.AxisListType.X, op=Alu.max)
            nc.vector.reciprocal(out=cc[0:1, 1:2], in_=cc[0:1, 0:1])
            rep_c = small.tile([1, P], f32, tag="rep_c")
            rep_i = small.tile([1, P], f32, tag="rep_i")
            nc.scalar.mul(out=rep_c[:], in_=cc[0:1, 0:1].to_broadcast((1, P)),
                          mul=0.2)
            nc.scalar.mul(out=rep_i[:], in_=cc[0:1, 1:2].to_broadcast((1, P)),
                          mul=2.5)
            cb = small.tile([P, 2], f32, tag="cb")
            nc.scalar.dma_start(out=cb[:, 0:1], in_=rep_c[:])
            nc.scalar.dma_start(out=cb[:, 1:2], in_=rep_i[:])
            st["cb"] = cb

        def pass2(i, st):
            # bh[:, n] = sum(ad + relu(ad - c)^2 * i2c)
            n = i % NLOC
            junk = work.tile([P, F], f32, tag="junk")
            nc.vector._custom_dve(berhu_op, out=junk[:], in0=st["ad"][:],
                                  s0=st["cb"][:, 0:1], s1=st["cb"][:, 1:2],
                                  accum_out=bh_t[:, n:n + 1])

        # 2-deep software pipeline: pass1(i) | chain(i-1) | pass2(i-2) keeps
        # the DVE stream free of waits on the c-derivation chain.
        hist = {}
        for i in range(total):
            hist[i] = load_pass1(i)
            if i - 1 >= 0:
                chain(hist[i - 1])
            if i - 2 >= 0:
                pass2(i - 2, hist.pop(i - 2))
        for i in (total - 2, total - 1):
            if i >= 0:
                if "cb" not in hist[i]:
                    chain(hist[i])
                pass2(i, hist.pop(i))

        nc.sync.dma_start(out=bh_d[:], in_=bh_t[:])

    nc.compile()
    return nc


def _get_prog():
    global _PROG
    if _PROG is None:
        _PROG = _build()
    return _PROG


def _combine(results):
    total = 0.0
    for r in results:
        total += r["bh"].astype(np.float64).sum()
    return np.float32(total / (N * S))


def kernel(predictions, targets):
    from concourse.bass_utils import run_bass_kernel_spmd

    nc = _get_prog()
    p = np.ascontiguousarray(
        np.asarray(predictions, dtype=np.float32).reshape(NCORES, NLOC * P, F))
    t = np.ascontiguousarray(
        np.asarray(targets, dtype=np.float32).reshape(NCORES, NLOC * P, F))
    in_maps = [{"p": p[k], "t": t[k]} for k in range(NCORES)]
    res = run_bass_kernel_spmd(nc, in_maps, list(range(NCORES)))
    return _combine(res.results)
